# revision 42
# baseline (speedup 1.0000x reference)
"""Trainium2 Bass kernel for CRF negative log-likelihood (nn_CRF).

Strategy (v2):
  - data-parallel over batch: 8 cores x 16 sequences each.
  - forward algorithm via a SEGMENTED RANK-1 scan in the exp domain
    (K=128 segments of L=2 steps, two 52-tag decks packed TIGHT at
    partition rows 0:52 / 52:104, block-diagonal exp(transitions)
    weights so one matmul advances both decks).
  - the round-0 forward matmul is folded into the weights on host:
    Wfp = diag(colsum(Etil)) @ Etil, so the device runs ONE fwd matmul
    + ONE multiply per segment round instead of two.
  - emissions are per-column max-normalized on host (values in (0,1])
    and shipped as fp8e4m3 -- half the DMA bytes of bf16; the
    normalizers fold into the host-side log combine.
  - gold path: host folds emit+trans+end energies into a single fp8
    "valued one-hot" tensor G; the device reduces it with one
    tensor_tensor_reduce (no one-hot matmuls, no gpsimd multiplies).
  - engine plan: Tensor [f0,b0,f1,b1,A0,S0,A1,Da,S1,Db]; Vector
    [Af0,U1c0,Af1,dotsMa,U1c1,dotsMb,ttrG]; Scalar does only the
    PSUM->SBUF copies (its ACT-table preamble no longer blocks DMA
    triggers, which all live on the sync HWDGE queue + gpsimd SWDGE).
  - PSUM fits in 6 banks by rotating psS/psD tiles through the
    psf/psb pool tags after their producers die.
"""

import numpy as np

TAG = 52
START, STOP = TAG - 2, TAG - 1
B, S = 128, 256
NCORES = 8
BL = B // NCORES            # 16 sequences per core
L = 2                       # steps per segment
K = S // L                  # 128 segments
KH = K // 2                 # 64 segments per deck
COLS = KH * BL              # 1024 columns
CH = 512                    # chunk width (one PSUM bank)
P = 2 * TAG                 # 104 used partitions (two tight decks)
MGATE = 64.0                # mask gate constant (exp(-64) == 0)
DA = 496                    # dots chunk a width (pairs 1..31)
DB = 512                    # dots chunk b width (pairs 32..63)
STW = 2 * CH + DA + DB      # 2032: stage width [sums c0|c1 | dots a|b]

_CACHE: dict = {}


def _build_nc(debug: bool = False):
    import concourse.mybir as mybir
    import concourse.tile as tile
    from concourse import bacc

    f32 = mybir.dt.float32
    bf16 = mybir.dt.bfloat16
    fp8 = mybir.dt.float8e4
    AL = mybir.AluOpType

    nc = bacc.Bacc("TRN2", target_bir_lowering=False, debug=debug)

    # ---- external inputs (per-core shards, host-marshalled layouts) ----
    # exp-domain emissions, chunk-round-major: [D0c0 | D1c0 | D0c1 | D1c1]
    # one flat row per partition -> ONE dma_start, 104 x 2KB descriptors
    dpack = nc.dram_tensor("dpack", (P, 4 * CH), fp8, kind="ExternalInput")
    # weights, split so the first matmul's Wfp arrives on its own fast path:
    # cpackA = Wfp(104); cpackB = [Wb(104) | WA(104) | W2(2) | pad]
    cpackA = nc.dram_tensor("cpackA", (P, P), bf16, kind="ExternalInput")
    cpackB = nc.dram_tensor("cpackB", (P, 212), bf16, kind="ExternalInput")
    # gold: host-folded emit+trans+end values at gold-tag one-hot slots
    # (the two 832-col halves are pre-summed on host)
    gpack = nc.dram_tensor("gpack", (128, 832), fp8, kind="ExternalInput")

    # ---- external outputs ----
    # [sums c0 | sums c1 | dots a | dots b | gold scalar at col STW]
    out_scan = nc.dram_tensor("out_scan", (2, STW + 4), f32, kind="ExternalOutput")
    out_bA = nc.dram_tensor("out_bA", (TAG, BL), f32, kind="ExternalOutput")
    out_bU = nc.dram_tensor("out_bU", (TAG, BL), bf16, kind="ExternalOutput")

    with tile.TileContext(nc) as tc:
        with (
            tc.tile_pool(name="persist", bufs=1) as persist,
            tc.tile_pool(name="psum", bufs=2, space="PSUM") as psum,
            tc.tile_pool(name="psap", bufs=1, space="PSUM") as psap,
        ):
            CT = persist.tile([P, 316], bf16, name="CT", tag="CT")
            D01 = persist.tile([P, 4 * CH], fp8, name="D01", tag="D01")
            G = persist.tile([128, 832], fp8, name="G", tag="G")
            # input DMAs: D01 chunk halves on the sync HWDGE queue (c0 first
            # -- it gates the scan); Wfp + gold on the gpsimd SWDGE; the
            # remaining weights on the scalar HWDGE queue, whose descriptor
            # write runs before the lazy ACT table load
            nc.sync.dma_start(out=D01[:, 0 : 2 * CH], in_=dpack[:, 0 : 2 * CH])
            nc.gpsimd.dma_start(out=CT[:, 0:P], in_=cpackA[:, :])
            nc.scalar.dma_start(out=CT[:, P : P + 212], in_=cpackB[:, :])
            nc.sync.dma_start(
                out=D01[:, 2 * CH : 4 * CH], in_=dpack[:, 2 * CH : 4 * CH]
            )
            nc.gpsimd.dma_start(out=G, in_=gpack[:, :])

            Wfp = CT[:, 0:P]
            Wb = CT[:, P : 2 * P]
            WA = CT[:, 2 * P : 3 * P]
            W2 = CT[:, 3 * P : 3 * P + 2]

            def D0(c):
                return D01[:, 2 * c * CH : (2 * c + 1) * CH]

            def D1(c):
                return D01[:, (2 * c + 1) * CH : (2 * c + 2) * CH]

            Af = persist.tile([P, COLS], bf16, name="Af", tag="Af")
            U1 = persist.tile([P, COLS], bf16, name="U1", tag="U1")
            dotsM = persist.tile([P, COLS - BL], bf16, name="dotsM", tag="dotsM")
            scrapG = persist.tile([128, 832], bf16, name="scrapG", tag="scrapG")
            g2 = persist.tile([128, 1], f32, name="g2", tag="g2")
            ones128 = persist.tile([128, 1], f32, name="ones128", tag="ones128")
            stage = persist.tile([2, STW + 4], f32, name="stage", tag="stage")
            bAt = persist.tile([TAG, BL], f32, name="bAt", tag="bAt")
            with tc.high_priority():
                nc.vector.memset(ones128, 1.0)
                nc.gpsimd.memset(stage[0:2, STW : STW + 4], 0.0)
                # dummy ACT op: pulls the lazy ~1.3us ACT_TABLE_LOAD into
                # the DMA phase instead of blocking the first real copy
                nc.scalar.copy(
                    stage[0:1, STW + 2 : STW + 3], stage[0:1, STW + 1 : STW + 2]
                )

            # Per-engine static order matters: the tile scheduler fixes
            # each engine queue at compile time.
            # Tensor: f0, b0, A0, f1, b1, A1, S0, Da, S1, Db  (A0 fills the
            #   slot where Tensor would otherwise stall on D01-c1's receipt)
            # Vector: Af0, U1c0, Af1, dotsMa, U1c1, dots32, dotsMb, cpDb
            # Scalar: dummy, G-reduce, cpS0, bA, cpDa, cpS1, g-copy
            psf = []
            psb = []
            ps = psum.tile([P, CH], f32, name="psf_0", tag="psf")
            nc.tensor.matmul(ps, Wfp, D0(0), start=True, stop=True)
            psf.append(ps)
            ps = psum.tile([P, CH], f32, name="psb_0", tag="psb")
            nc.tensor.matmul(ps, Wb, D1(0), start=True, stop=True)
            psb.append(ps)
            nc.vector.tensor_tensor(
                out=Af[:, 0:CH], in0=psf[0], in1=D1(0), op=AL.mult
            )
            # psA as two single-bank tiles: a shared 2-bank tile makes the
            # dots ops falsely depend on BOTH psA matmuls (no PSUM subtiles)
            psA_a = psap.tile([P, CH], f32, name="psA_a", tag="psA_a")
            psA_b = psap.tile([P, CH], f32, name="psA_b", tag="psA_b")
            nc.tensor.matmul(psA_a, WA, Af[:, 0:CH], start=True, stop=True)
            nc.vector.tensor_tensor(
                out=U1[:, 0:CH], in0=psb[0], in1=D0(0), op=AL.mult
            )
            # deck-crossing boundary piece: U_64 (deck 1, first block)
            nc.sync.dma_start(out=out_bU[:, :], in_=U1[TAG : 2 * TAG, 0:BL])
            ps = psum.tile([P, CH], f32, name="psf_1", tag="psf")
            nc.tensor.matmul(ps, Wfp, D0(1), start=True, stop=True)
            psf.append(ps)
            ps = psum.tile([P, CH], f32, name="psb_1", tag="psb")
            nc.tensor.matmul(ps, Wb, D1(1), start=True, stop=True)
            psb.append(ps)
            nc.vector.tensor_tensor(
                out=Af[:, CH:COLS], in0=psf[1], in1=D1(1), op=AL.mult
            )
            nc.tensor.matmul(psA_b, WA, Af[:, CH:COLS], start=True, stop=True)
            # dots chunk a: pairs (a_j, b_{j+1}) entirely inside chunk 0
            nc.vector.tensor_tensor(
                out=dotsM[:, 0:DA],
                in0=psA_a[:, 0:DA],
                in1=U1[:, BL : BL + DA],
                op=AL.mult,
            )
            nc.vector.tensor_tensor(
                out=U1[:, CH:COLS], in0=psb[1], in1=D0(1), op=AL.mult
            )
            # pair 32 (chunk boundary), then pairs 33..63
            nc.vector.tensor_tensor(
                out=dotsM[:, DA:CH],
                in0=psA_a[:, DA:CH],
                in1=U1[:, CH : CH + BL],
                op=AL.mult,
            )
            nc.vector.tensor_tensor(
                out=dotsM[:, CH : COLS - BL],
                in0=psA_b[:, 0:DA],
                in1=U1[:, CH + BL : COLS],
                op=AL.mult,
            )
            psS0 = psum.tile([2, CH], f32, name="psS0", tag="psb")
            nc.tensor.matmul(psS0, W2, Af[:, 0:CH], start=True, stop=True)
            nc.scalar.copy(stage[0:2, 0:CH], psS0)
            # boundary: Etil^T a_63 (deck 0, last block of chunk 1)
            nc.scalar.copy(bAt, psA_b[0:TAG, CH - BL : CH])
            nc.sync.dma_start(out=out_bA[:, :], in_=bAt)
            psDa = psum.tile([2, CH], f32, name="psDa", tag="psf")
            nc.tensor.matmul(
                psDa[:, 0:DA], W2, dotsM[:, 0:DA], start=True, stop=True
            )
            nc.scalar.copy(stage[0:2, 2 * CH : 2 * CH + DA], psDa[:, 0:DA])
            psS1 = psum.tile([2, CH], f32, name="psS1", tag="psb")
            nc.tensor.matmul(psS1, W2, Af[:, CH:COLS], start=True, stop=True)
            nc.scalar.copy(stage[0:2, CH : 2 * CH], psS1)
            psDb = psum.tile([2, CH], f32, name="psDb", tag="psf")
            nc.tensor.matmul(
                psDb, W2, dotsM[:, DA : DA + DB], start=True, stop=True
            )
            # ship the early staged outputs while psDb is still copying, so
            # the last DMA (and its ~2us HBM write receipt) is tiny and early
            nc.sync.dma_start(
                out=out_scan[:, 0 : 2 * CH + DA], in_=stage[:, 0 : 2 * CH + DA]
            )
            # gold: Scalar-engine copy-with-accumulate reduces G along the
            # free axis (keeps the Vector queue clear), then a single fp32
            # ones-matmul folds the 128 partials into one stage scalar
            with tc.high_priority(offset=-100000):
                nc.scalar.activation(
                    out=scrapG,
                    in_=G,
                    func=mybir.ActivationFunctionType.Copy,
                    accum_out=g2,
                )
                psG = psum.tile([1, 1], f32, name="psG", tag="psG", bufs=1)
                nc.tensor.matmul(psG, ones128, g2, start=True, stop=True)
                nc.scalar.copy(stage[0:1, STW : STW + 1], psG)
            nc.vector.tensor_copy(stage[0:2, 2 * CH + DA : STW], psDb)
            nc.sync.dma_start(
                out=out_scan[:, 2 * CH + DA :], in_=stage[:, 2 * CH + DA :]
            )

    nc.compile()
    return nc


def _prep_core_inputs(feats, transitions, mask, tags, core):
    """Layout-only host marshalling of the core's batch shard."""
    f32 = np.float32
    import ml_dtypes

    bf16 = ml_dtypes.bfloat16
    fp8 = ml_dtypes.float8_e4m3
    sl = slice(core * BL, (core + 1) * BL)
    f = np.ascontiguousarray(feats[sl]).astype(f32, copy=False)   # (BL,S,T)
    m = mask[sl].astype(f32)                                      # (BL,S)
    tg = np.clip(tags[sl].astype(np.int64), 0, TAG - 1)           # (BL,S)

    tc = transitions.astype(f32).copy()
    tc[STOP, STOP] = 0.0                                          # exp -> 1
    et = np.exp(tc)
    cs = et.sum(axis=0)

    # masked/gated log-emissions (absorbing-STOP construction), C0 = 0
    g = f.transpose(2, 1, 0).copy()                               # (T,S,BL)
    g[STOP] = 0.0
    act = (m.T > 0)[None, :, :]                                   # (1,S,BL)
    rowstop = np.zeros((TAG, 1, 1), bool)
    rowstop[STOP] = True
    g = np.where(
        act,
        np.where(rowstop, -MGATE, g),
        np.where(rowstop, 0.0, -MGATE),
    ).astype(f32)
    # fold chain inits into the emissions so every chain starts from ones:
    #  t=0 (seg0 fwd):  + log Etil[START,:] - log colsum(Etil)
    #  t=S-1 (segK-1 bwd): + log Etil[:,STOP]
    corr = np.where(cs > 0, tc[START, :] - np.log(np.maximum(cs, 1e-30)), 0.0)
    g[:, 0, :] += corr.astype(f32)[:, None]
    g[:, S - 1, :] += tc[:, STOP][:, None]
    # per-column max normalization -> exp values in (0,1], fp8-safe
    mnorm = g.max(axis=0)                                         # (S,BL)
    eg = np.exp(g - mnorm[None])                                  # (T,S,BL)
    gr = eg.reshape(TAG, K, L, BL)
    dpack = np.zeros((4, P, CH), f32)
    for c in range(2):
        for r in range(L):
            for d in range(2):
                blk = gr[:, d * KH + c * 32 : d * KH + (c + 1) * 32, r, :]
                dpack[2 * c + r, TAG * d : TAG * (d + 1), :] = blk.reshape(
                    TAG, CH
                )
    dpack = np.ascontiguousarray(dpack.transpose(1, 0, 2)).reshape(P, 4 * CH)

    # weights: cpackA = Wfp; cpackB = [Wb | WA | W2 | pad] (tight decks)
    cpackA = np.zeros((P, P), f32)
    cpackB = np.zeros((P, 212), f32)
    for d in range(2):
        r0 = TAG * d
        cpackA[r0 : r0 + TAG, r0 : r0 + TAG] = cs[:, None] * et
        cpackB[r0 : r0 + TAG, r0 : r0 + TAG] = et.T
        cpackB[r0 : r0 + TAG, P + r0 : P + r0 + TAG] = et
        cpackB[r0 : r0 + TAG, 2 * P + d] = 1.0

    # gold: emit+trans+end energies folded into gold-tag one-hot slots
    prev = np.concatenate([np.full((BL, 1), START, np.int64), tg[:, :-1]], 1)
    mnext = np.concatenate([m[:, 1:], np.zeros((BL, 1), f32)], axis=1)
    wl = m - mnext                                                # last active
    bi = np.arange(BL)[:, None]
    ti = np.arange(S)[None, :]
    gval = (f[bi, ti, tg] + tc[prev, tg] + wl * tc[tg, STOP]) * m
    G3 = np.zeros((BL, S, TAG), f32)
    G3[bi, ti, tg] = gval
    Gp = G3.reshape(4096, TAG).reshape(128, 32 * TAG)             # (128,1664)
    gpack = Gp[:, 0:832] + Gp[:, 832:1664]                        # (128,832)

    in_map = {
        "dpack": dpack.astype(fp8),
        "cpackA": cpackA.astype(bf16),
        "cpackB": cpackB.astype(bf16),
        "gpack": gpack.astype(fp8),
    }
    return in_map, mnorm.sum(axis=0).astype(np.float64)


def _combine(results, msums):
    """Host-side unshard: logs of staged dots/sums + gold partials."""
    fwd = np.float64(0.0)
    gold = np.float64(0.0)
    for core, res in enumerate(results):
        sc = res["out_scan"].astype(np.float64)                   # (2, STW+4)
        bA = res["out_bA"].astype(np.float64)                     # (52, BL)
        bU = res["out_bU"].astype(np.float64)                     # (52, BL)
        sums = sc[:, 0 : 2 * CH]                                  # (2,1024)
        dots = sc[:, 2 * CH : STW]                                # (2,1008)
        s0 = sums[0].reshape(KH, BL)[1:KH]                        # s_1..63
        s1 = sums[1].reshape(KH, BL)[0 : KH - 1]                  # s_64..126
        d0 = dots[0].reshape(KH - 1, BL)                          # dot_1..63
        d1 = dots[1].reshape(KH - 1, BL)                          # dot_65..127
        dot64 = (bA * bU).sum(axis=0)                             # (BL,)
        fwd_core = (
            np.log(d0).sum(axis=0)
            + np.log(d1).sum(axis=0)
            + np.log(dot64)
            - np.log(s0).sum(axis=0)
            - np.log(s1).sum(axis=0)
            + msums[core]
        )
        fwd += fwd_core.sum()
        gold += sc[0, STW]
    return np.asarray(fwd - gold, dtype=np.float32)[()]


def kernel(feats, transitions, mask, tags):
    feats = np.asarray(feats)
    transitions = np.asarray(transitions)
    mask = np.asarray(mask)
    tags = np.asarray(tags)

    if "nc" not in _CACHE:
        _CACHE["nc"] = _build_nc(debug=False)
    nc = _CACHE["nc"]

    from concourse import bass_utils

    preps = [
        _prep_core_inputs(feats, transitions, mask, tags, c)
        for c in range(NCORES)
    ]
    in_maps = [p[0] for p in preps]
    msums = [p[1] for p in preps]
    out = bass_utils.run_bass_kernel_spmd(nc, in_maps, core_ids=list(range(NCORES)))
    return _combine(out.results, msums)
